# revision 49
# baseline (speedup 1.0000x reference)
"""Distributed GCN (2-layer + readout) on 8 Trainium2 NeuronCores.

Gather + two-level on-chip reduce (no per-edge scatter):

Nodes are sharded 8-way by dst owner. Per GCN layer each core builds its
table shard s = dinv * (h @ W) on TensorE; per-half AllGathers
materialize the full table in every core's HBM (4 chunks, one per
(half, core-quad); local row 0 and row HALF of every shard are reserved
ZERO rows so each chunk has a zero row at idx16 == 0 for padding).

Messages are pre-sorted host-side into 4 chunk-pure sections. Within a
section, each (dst, chunk) subrun is padded to a bucket length K and
assigned a (partition, K-column) rectangle, so gpsimd.dma_gather lands
all messages of a subrun in ONE partition across K consecutive columns.
Level-1: a strided DVE tensor_reduce sums each subrun -> partial rows,
DMA'd to a DRAM staging table P. Level-2: a uniform K=2 combine-gather
reads each dst's two partials per chunk-pair from P, reduces, and
accumulates into the table tile (self-loop term), followed by the
epilogue relu(dinv * (.) + b). This removes all dma_scatter_add traffic
(~53ns/descriptor vs ~17ns for gathers) and the WAW serialization that
dominated the scatter-based kernel.
"""
import numpy as np
from collections import defaultdict

from concourse import bass, bacc, tile, mybir, bass_utils

F32 = mybir.dt.float32
I16 = mybir.dt.int16

NCORES = 8
D = 64
BUCKETS = [1, 2, 3, 4, 5, 6, 8, 10, 12, 16, 24, 32, 48, 64]
REGION_COLS = 16


def _roundup(x, m):
    return (x + m - 1) // m * m


def preprocess(edge_index, n_nodes):
    src = np.asarray(edge_index[0], dtype=np.int64)
    dst = np.asarray(edge_index[1], dtype=np.int64)

    REAL = (n_nodes + NCORES - 1) // NCORES
    NL = _roundup(REAL + 2, 256)
    HALF = NL // 2
    CHUNK = NL * 2
    G = NL // 128
    GH = G // 2

    def rho_of(loc):
        return np.where(loc < HALF - 1, loc + 1, loc + 2)

    # table storage is partition-major per core-half: slot = p*GH + g
    # (so the ag_in DMA from SBUF [128, GH, 64] is contiguous per partition)
    s_owner = src // REAL
    s_rho = rho_of(src % REAL)
    s_h = (s_rho >= HALF).astype(np.int64)
    rh = s_rho % HALF
    slot = (rh % 128) * GH + rh // 128
    pos_half = s_owner * HALF + slot
    s_j2 = pos_half // CHUNK
    sec = s_h * 2 + s_j2
    idx16 = (pos_half % CHUNK).astype(np.int64)
    assert idx16.max() < 32768

    d_owner = dst // REAL
    d_rho = rho_of(dst % REAL)

    degcnt = np.bincount(dst, minlength=n_nodes).astype(np.float32) + 1.0
    deg_tiles = []
    for c in range(NCORES):
        # reserved/pad rows get deg=1e30 -> dinv~1e-15, zeroing their table
        # rows (s = dinv * hW) so chunk zero rows / pad gathers stay ~0.
        d = np.full(NL, 1e30, np.float32)
        lo, hi = c * REAL, min((c + 1) * REAL, n_nodes)
        loc = np.arange(hi - lo)
        d[rho_of(loc)] = degcnt[lo:hi]
        deg_tiles.append(np.ascontiguousarray(d.reshape(G, 128).T))

    subruns = [[defaultdict(list) for _ in range(4)] for _ in range(NCORES)]
    order = np.lexsort((idx16, d_rho, sec, d_owner))
    so, ss, sr, si = d_owner[order], sec[order], d_rho[order], idx16[order]
    key = (so * 4 + ss) * NL + sr
    bounds = np.flatnonzero(np.diff(key)) + 1
    starts = np.concatenate([[0], bounds])
    ends = np.concatenate([bounds, [len(key)]])
    maxlen = 0
    for a, b in zip(starts, ends):
        c, s, r = int(so[a]), int(ss[a]), int(sr[a])
        subruns[c][s][r] = si[a:b]
        maxlen = max(maxlen, b - a)
    assert maxlen <= BUCKETS[-1]

    def bucket_of(n):
        for K in BUCKETS:
            if n <= K:
                return K
        raise AssertionError

    nseg = np.zeros((4, len(BUCKETS)), np.int64)
    for c in range(NCORES):
        for j in range(4):
            cnt = defaultdict(int)
            for r, lst in subruns[c][j].items():
                cnt[bucket_of(len(lst))] += 1
            for ki, K in enumerate(BUCKETS):
                nseg[j, ki] = max(nseg[j, ki], cnt[K])

    col = 0
    sec_info = []
    for j in range(4):
        info = []
        prow = 0
        for ki, K in enumerate(BUCKETS):
            n = int(nseg[j, ki])
            if n == 0:
                continue
            m = (n + 127) // 128
            info.append(dict(K=K, m=m, col0=col, prow0=prow))
            col += m * K
            prow += m * 128
        sec_info.append(info)
    TOTCOLS = col
    TOTPOS = TOTCOLS * 128
    # P is partition-major: [128, 1 + PM, 64]; row idx = p*(1+PM) + gcol;
    # gcol 0 is the reserved zero column. PMSEC[j] = g-columns of section j.
    PMSEC = [sum(i["m"] for i in sec_info[j]) for j in range(4)]
    PMA = 1 + PMSEC[0] + PMSEC[1]
    PMB = 1 + PMSEC[2] + PMSEC[3]
    CA = 128 * PMA
    CB = 128 * PMB
    assert CA <= 32768 and CB <= 32768, (CA, CB)
    # g-column base of each section within its chunk tensor
    sec_gbase = [1, 1 + PMSEC[0], 1, 1 + PMSEC[2]]

    regions = []
    max_mtot = 0
    for j in range(4):
        pend = []
        for i in sec_info[j]:
            for m in range(i["m"]):
                pend.append((i["K"], i["col0"] + m * i["K"],
                             i["prow0"] + m * 128))
        r0 = 0
        while r0 < len(pend):
            c0 = pend[r0][1]
            ncols = 0
            r1 = r0
            while r1 < len(pend) and ncols + pend[r1][0] <= REGION_COLS:
                ncols += pend[r1][0]
                r1 += 1
            assert r1 > r0
            reds = []
            q = r0
            while q < r1:
                K = pend[q][0]
                mcount = 0
                qq = q
                while qq < r1 and pend[qq][0] == K:
                    mcount += 1
                    qq += 1
                reds.append((K, pend[q][1] - c0, mcount))
                q = qq
            mtot = sum(r[2] for r in reds)
            max_mtot = max(max_mtot, sum(r[2] for r in reds if r[0] > 1))
            regions.append(dict(sec=j, chunk=j // 2, col0=c0, ncols=ncols,
                                reds=reds, mtot=mtot,
                                goff=sec_gbase[j] + pend[r0][2] // 128))
            r0 = r1

    gidx_all = []
    prow_of = []
    for c in range(NCORES):
        vals = np.zeros(TOTPOS, np.int16)
        pmap = [dict() for _ in range(4)]
        for j in range(4):
            byK = defaultdict(list)
            for r in sorted(subruns[c][j].keys()):
                lst = subruns[c][j][r]
                byK[bucket_of(len(lst))].append((r, lst))
            for i in sec_info[j]:
                K = i["K"]
                segs = byK.get(K, [])
                assert len(segs) <= i["m"] * 128
                PM = PMA if j < 2 else PMB
                for s, (r, lst) in enumerate(segs):
                    m, p = s // 128, s % 128
                    colb = i["col0"] + m * K
                    for k, v in enumerate(lst):
                        vals[(colb + k) * 128 + p] = v
                    gcol = sec_gbase[j] + i["prow0"] // 128 + m
                    pmap[j][r] = p * PM + gcol
        gw = np.ascontiguousarray(np.tile(vals.reshape(-1, 16).T, (8, 1)))
        gidx_all.append(gw)
        prow_of.append(pmap)

    # quarters aligned to half boundaries: q0,q1 cover half 0; q2,q3 half 1
    qa = (GH + 1) // 2
    GQ = [qa, GH - qa, qa, GH - qa]
    q_g0 = list(np.cumsum([0] + GQ)[:-1])
    CCOLS = 2 * G * 2
    CPOS = CCOLS * 128
    cidx_all = []
    for c in range(NCORES):
        vals = np.zeros(CPOS, np.int16)
        colbase = 0
        for ab in range(2):
            for q in range(4):
                g0 = q_g0[q]
                for gg in range(GQ[q]):
                    for m in range(2):
                        j = ab * 2 + m
                        colc = colbase + 2 * gg + m
                        pm = prow_of[c][j]
                        for p in range(128):
                            rho = (g0 + gg) * 128 + p
                            vals[colc * 128 + p] = pm.get(rho, 0)
                colbase += 2 * GQ[q]
        cw = np.ascontiguousarray(np.tile(vals.reshape(-1, 16).T, (8, 1)))
        cidx_all.append(cw)

    # interleave regions across the two sections of each half so
    # consecutive gathers hit different chunks (better engine pipelining)
    def _ileave(a, b):
        out = []
        for i in range(max(len(a), len(b))):
            if i < len(a):
                out.append(a[i])
            if i < len(b):
                out.append(b[i])
        return out
    bysec = [[r for r in regions if r["sec"] == j] for j in range(4)]
    regions = _ileave(bysec[0], bysec[1]) + _ileave(bysec[2], bysec[3])

    meta = dict(REAL=REAL, NL=NL, HALF=HALF, CHUNK=CHUNK, G=G, GH=GH,
                TOTCOLS=TOTCOLS, TOTPOS=TOTPOS, regions=regions,
                CA=CA, CB=CB, PMA=PMA, PMB=PMB, GQ=GQ, q_g0=q_g0,
                CCOLS=CCOLS, CPOS=CPOS, max_mtot=max_mtot)
    return meta, gidx_all, cidx_all, deg_tiles


def build(meta):
    NL, HALF, CHUNK, G, GH = (meta["NL"], meta["HALF"], meta["CHUNK"],
                              meta["G"], meta["GH"])
    REAL = meta["REAL"]
    regions = meta["regions"]
    TOTPOS, CPOS = meta["TOTPOS"], meta["CPOS"]
    CA, CB = meta["CA"], meta["CB"]
    PMA, PMB = meta["PMA"], meta["PMB"]
    GQ, q_g0 = meta["GQ"], meta["q_g0"]
    MMAX = max(meta["max_mtot"], 1)

    nc = bacc.Bacc("TRN2", target_bir_lowering=False, debug=False,
                   num_devices=NCORES, num_swdge_queues=4)

    xT = nc.dram_tensor("xT", [D, NL], F32, kind="ExternalInput")
    W1 = nc.dram_tensor("W1", [D, D], F32, kind="ExternalInput")
    W2 = nc.dram_tensor("W2", [D, D], F32, kind="ExternalInput")
    b1e = nc.dram_tensor("b1bc", [128, D], F32, kind="ExternalInput")
    b2e = nc.dram_tensor("b2bc", [128, D], F32, kind="ExternalInput")
    woute = nc.dram_tensor("woutbc", [128, D], F32, kind="ExternalInput")
    boute = nc.dram_tensor("boutbc", [128, 1], F32, kind="ExternalInput")
    dege = nc.dram_tensor("deg", [128, G], F32, kind="ExternalInput")
    gidxe = nc.dram_tensor("gidx", [128, TOTPOS // 16], I16,
                           kind="ExternalInput")
    cidxe = nc.dram_tensor("cidx", [128, CPOS // 16], I16,
                           kind="ExternalInput")
    idente = nc.dram_tensor("ident", [128, 128], F32, kind="ExternalInput")
    oute = nc.dram_tensor("out", [128, G], F32, kind="ExternalOutput")

    ag_in = [[nc.dram_tensor(f"ag_in{L}_{h}", [HALF, D], F32)
              for h in (0, 1)] for L in (0, 1)]
    ag_out = [[nc.dram_tensor(f"ag_out{L}_{h}", [NCORES * HALF, D], F32,
                              addr_space="Shared")
               for h in (0, 1)] for L in (0, 1)]
    PA = [nc.dram_tensor(f"PA{L}", [CA, D], F32) for L in (0, 1)]
    PB = [nc.dram_tensor(f"PB{L}", [CB, D], F32) for L in (0, 1)]

    def pmaj_view(dram, cols):
        # partition-major: row = p*cols + g -> [128, cols, 64]
        return dram.ap().rearrange("(p g) d -> p g d", g=cols)

    with tile.TileContext(nc) as tc:
        with (
            tc.tile_pool(name="pool", bufs=1) as pool,
            tc.tile_pool(name="xs", bufs=2) as xspool,
            tc.tile_pool(name="msg", bufs=12) as msgpool,
            tc.tile_pool(name="red", bufs=3) as redpool,
            tc.tile_pool(name="cm", bufs=2) as cmpool,
            tc.tile_pool(name="fence", bufs=2) as fencepool,
            tc.tile_pool(name="psum", bufs=2, space="PSUM") as psum,
        ):
            gidx_t = pool.tile([128, TOTPOS // 16], I16, tag="gidx")
            cidx_t = pool.tile([128, CPOS // 16], I16, tag="cidx")
            nc.scalar.dma_start(out=gidx_t[:], in_=gidxe[:])
            nc.scalar.dma_start(out=cidx_t[:], in_=cidxe[:])
            W1_t = pool.tile([D, D], F32, tag="w1")
            W2_t = pool.tile([D, D], F32, tag="w2")
            nc.scalar.dma_start(out=W1_t[:], in_=W1[:])
            nc.scalar.dma_start(out=W2_t[:], in_=W2[:])
            b1_t = pool.tile([128, D], F32, tag="b1")
            b2_t = pool.tile([128, D], F32, tag="b2")
            wout_t = pool.tile([128, D], F32, tag="wout")
            bout_t = pool.tile([128, 1], F32, tag="bout")
            ident_t = pool.tile([128, 128], F32, tag="ident")
            nc.scalar.dma_start(out=b1_t[:], in_=b1e[:])
            nc.scalar.dma_start(out=b2_t[:], in_=b2e[:])
            nc.scalar.dma_start(out=wout_t[:], in_=woute[:])
            nc.scalar.dma_start(out=bout_t[:], in_=boute[:])
            nc.scalar.dma_start(out=ident_t[:], in_=idente[:])
            deg_t = pool.tile([128, G], F32, tag="deg")
            nc.sync.dma_start(out=deg_t[:], in_=dege[:])
            dinv_t = pool.tile([128, G], F32, tag="dinv")
            nc.scalar.activation(dinv_t[:], deg_t[:],
                                 mybir.ActivationFunctionType.Sqrt)
            nc.vector.reciprocal(dinv_t[:], dinv_t[:])

            # zero g-column 0 of P chunks (combine pad target)
            z_t = pool.tile([128, 1, D], F32, tag="zrow")
            nc.vector.memset(z_t[:], 0.0)
            for L in (0, 1):
                nc.scalar.dma_start(out=pmaj_view(PA[L], PMA)[:, 0:1, :],
                                    in_=z_t[:])
                nc.scalar.dma_start(out=pmaj_view(PB[L], PMB)[:, 0:1, :],
                                    in_=z_t[:])

            # --- layer-1 table: s1 = dinv * (x @ W1), per half + AllGather
            s1_t = pool.tile([128, G, D], F32, tag="s1")
            for h in (0, 1):
                g0, g1 = h * GH, (h + 1) * GH
                for t0 in range(g0, g1, 8):
                    nt = min(8, g1 - t0)
                    xT_t = xspool.tile([D, 8 * 128], F32, tag="xT")
                    nc.sync.dma_start(out=xT_t[:, :nt * 128],
                                      in_=xT[:, t0 * 128:(t0 + nt) * 128])
                    pt = psum.tile([128, 512], F32, tag="mm")
                    for t in range(t0, t0 + nt):
                        nc.tensor.matmul(
                            pt[:, (t - t0) * D:(t - t0 + 1) * D],
                            xT_t[:, (t - t0) * 128:(t - t0 + 1) * 128],
                            W1_t[:])
                    for t in range(t0, t0 + nt):
                        nc.vector.tensor_scalar_mul(
                            s1_t[:, t, :],
                            pt[:, (t - t0) * D:(t - t0 + 1) * D],
                            dinv_t[:, t:t + 1])
                nc.sync.dma_start(out=pmaj_view(ag_in[0][h], GH),
                                  in_=s1_t[:, g0:g1, :])
                nc.gpsimd.collective_compute(
                    "AllGather", mybir.AluOpType.bypass,
                    replica_groups=[list(range(NCORES))],
                    ins=[ag_in[0][h].ap().opt()],
                    outs=[ag_out[0][h].ap().opt()])

            s2_t = pool.tile([128, G, D], F32, tag="s2")
            dma_sems = [nc.alloc_semaphore(f"gdma{q}") for q in range(16)]
            qcnt = [0] * 16

            def gather_pt(out_ap, in_ap, idxs_ap, n, qn, ph):
                """prepare_only gather + per-instruction trigger: desc-gen
                on gpsimd, drain decoupled (engine not held for the drain).
                Tile does NOT auto-wait for the drain on consumers; callers
                must wait_ge(dma_sems[qn], ret) on the consumer engine."""
                nc.gpsimd.dma_gather(
                    out_ap, in_ap, idxs_ap,
                    num_idxs=n, num_idxs_reg=n, elem_size=D,
                    single_packet=False, queue_num=qn,
                    prepare_only=True, sem=dma_sems[ph * 4 + qn])
                nc.gpsimd.trigger_dma(count=None, queue_num=qn)
                qcnt[ph * 4 + qn] += 1
                return ph * 4 + qn, 16 * qcnt[ph * 4 + qn]

            def fence_gather(in_ap, idxs_ap):
                """Tiny NORMAL-mode gather: holds the in-order gpsimd engine
                until in_ap's producers complete (the proven dependency
                path), so later prepare/trigger drains can't race them."""
                ft = fencepool.tile([128, 1, D], F32, tag="f")
                nc.gpsimd.dma_gather(
                    ft[:], in_ap, idxs_ap,
                    num_idxs=128, num_idxs_reg=128, elem_size=D,
                    single_packet=False, queue_num=0)

            def gs_layer(L):
                """Gather regions + level-1 reduce -> PA/PB[L]."""
                seen_sec = set()
                for i, reg in enumerate(regions):
                    j = reg["sec"]
                    h, j2 = j // 2, j % 2
                    Pd = PA[L] if reg["chunk"] == 0 else PB[L]
                    ncols = reg["ncols"]
                    c0 = reg["col0"]
                    mt = msgpool.tile([128, REGION_COLS, D], F32, tag="m")
                    qn = i % 4
                    nc.gpsimd.dma_gather(
                        mt[:, :ncols, :],
                        ag_out[L][h][j2 * CHUNK:(j2 + 1) * CHUNK, :],
                        gidx_t[:, c0 * 8:(c0 + ncols) * 8],
                        num_idxs=ncols * 128, num_idxs_reg=ncols * 128,
                        elem_size=D, single_packet=False, queue_num=qn)
                    PM = PMA if reg["chunk"] == 0 else PMB
                    Pv = pmaj_view(Pd, PM)
                    goff = reg["goff"]
                    # leading K=1 groups: ship partials straight from mt
                    reds = reg["reds"]
                    if reds and reds[0][0] == 1:
                        m1 = reds[0][2]
                        nc.sync.dma_start(out=Pv[:, goff:goff + m1, :],
                                          in_=mt[:, :m1, :])
                        goff += m1
                        reds = reds[1:]
                    if not reds:
                        continue
                    pt = redpool.tile([128, MMAX, D], F32, tag="pt")
                    o = 0
                    for (K, crel, mcount) in reds:
                        v = mt[:, crel:crel + mcount * K, :].rearrange(
                            "p (m k) d -> p m d k", k=K)
                        nc.vector.tensor_reduce(
                            pt[:, o:o + mcount, :], v,
                            axis=mybir.AxisListType.X,
                            op=mybir.AluOpType.add)
                        o += mcount
                    nc.sync.dma_start(out=Pv[:, goff:goff + o, :],
                                      in_=pt[:, :o, :])

            CQ = max(GQ)

            def combine_q(L, ab, q, s_t, qi):
                """Gather quarter q of chunk ab's partials, add into s_t."""
                gq, g0 = GQ[q], q_g0[q]
                colbase = ab * 2 * G + 2 * q_g0[q]
                n = 2 * gq * 128
                pch = PA[L] if ab == 0 else PB[L]
                cm = cmpool.tile([128, 2 * CQ, D], F32, tag="cm")
                qn = qi % 4
                nc.gpsimd.dma_gather(
                    cm[:, :2 * gq, :], pch[:],
                    cidx_t[:, colbase * 8:(colbase + 2 * gq) * 8],
                    num_idxs=n, num_idxs_reg=n, elem_size=D,
                    single_packet=False, queue_num=qn)
                av = s_t[:, g0:g0 + gq, :]
                for m in (0, 1):
                    vm = cm[:, m:2 * gq:2, :]
                    nc.vector.tensor_tensor(av, av, vm,
                                            mybir.AluOpType.add)

            def epilogue_q(s_t, bias_t, q):
                """In place: quarter q of s_t <- relu(dinv * s_t + b)."""
                gq, g0 = GQ[q], q_g0[q]
                av = s_t[:, g0:g0 + gq, :]
                dvb = dinv_t[:, g0:g0 + gq].unsqueeze(
                    2).broadcast_to([128, gq, D])
                nc.vector.tensor_tensor(av, av, dvb, mybir.AluOpType.mult)
                bb = bias_t[:].unsqueeze(1).broadcast_to([128, gq, D])
                nc.vector.tensor_tensor(av, av, bb, mybir.AluOpType.add)
                nc.scalar.activation(av, av,
                                     mybir.ActivationFunctionType.Relu)

            def table2_q(q):
                """Quarter q of layer-2 table: s2 = dinv * (a1 @ W2)."""
                gq, g0 = GQ[q], q_g0[q]
                for t in range(g0, g0 + gq):
                    tp = psum.tile([64, 128], F32, tag="tr")
                    nc.tensor.transpose(tp[:], s1_t[:, t, :], ident_t[:])
                    a1T = pool.tile([64, 128], F32, tag="a1T")
                    nc.vector.tensor_copy(a1T[:], tp[:])
                    pt2 = psum.tile([128, D], F32, tag="mm2")
                    nc.tensor.matmul(pt2[:], a1T[:], W2_t[:])
                    nc.vector.tensor_scalar_mul(s2_t[:, t, :], pt2[:],
                                                dinv_t[:, t:t + 1])

            def layer_back_half(L, s_t, bias_t):
                """combine A+B, epilogue; L=0 also builds + AllGathers s2."""
                qi = 0
                for q in range(4):
                    combine_q(L, 0, q, s_t, qi)
                    qi += 1
                for q in range(4):
                    combine_q(L, 1, q, s_t, qi)
                    qi += 1
                    epilogue_q(s_t, bias_t, q)
                    if L == 0:
                        table2_q(q)
                        if q == 1 or q == 3:
                            h = q // 2
                            nc.sync.dma_start(
                                out=pmaj_view(ag_in[1][h], GH),
                                in_=s2_t[:, h * GH:(h + 1) * GH, :])
                            nc.gpsimd.collective_compute(
                                "AllGather", mybir.AluOpType.bypass,
                                replica_groups=[list(range(NCORES))],
                                ins=[ag_in[1][h].ap().opt()],
                                outs=[ag_out[1][h].ap().opt()])

            gs_layer(0)
            layer_back_half(0, s1_t, b1_t)   # a1 in s1_t; s2 built + AG'd
            gs_layer(1)
            layer_back_half(1, s2_t, b2_t)   # a2 in s2_t

            # --- readout: out = (a2 * WoutT).sum(d) + bout ---
            o_t = pool.tile([128, G], F32, tag="o")
            wb = wout_t[:].unsqueeze(1).broadcast_to([128, G, D])
            nc.vector.tensor_tensor(s2_t[:], s2_t[:], wb,
                                    mybir.AluOpType.mult)
            nc.vector.tensor_reduce(o_t[:], s2_t[:],
                                    axis=mybir.AxisListType.X,
                                    op=mybir.AluOpType.add)
            nc.vector.tensor_scalar_add(o_t[:], o_t[:], bout_t[:])
            nc.sync.dma_start(out=oute[:], in_=o_t[:])

    nc.compile()
    return nc


_CACHE = {}


def kernel(x, edge_index, batch, W1, b1, W2, b2, Wout, bout, _trace=False):
    x = np.asarray(x, np.float32)
    edge_index = np.asarray(edge_index)
    W1 = np.asarray(W1, np.float32)
    W2 = np.asarray(W2, np.float32)
    b1 = np.asarray(b1, np.float32)
    b2 = np.asarray(b2, np.float32)
    Wout = np.asarray(Wout, np.float32)
    bout = np.asarray(bout, np.float32).reshape(-1)
    N = x.shape[0]

    key = (N, edge_index.shape[1])
    if key not in _CACHE:
        meta, gidx_all, cidx_all, deg_tiles = preprocess(edge_index, N)
        nc = build(meta)
        _CACHE[key] = (meta, gidx_all, cidx_all, deg_tiles, nc)
    meta, gidx_all, cidx_all, deg_tiles, nc = _CACHE[key]
    REAL, NL, HALF = meta["REAL"], meta["NL"], meta["HALF"]

    ident = np.eye(128, dtype=np.float32)
    b1bc = np.tile(b1[None, :], (128, 1)).astype(np.float32)
    b2bc = np.tile(b2[None, :], (128, 1)).astype(np.float32)
    woutbc = np.tile(Wout.reshape(1, -1), (128, 1)).astype(np.float32)
    boutbc = np.full((128, 1), float(bout[0]), np.float32)

    in_maps = []
    for c in range(NCORES):
        xs = np.zeros((NL, D), np.float32)
        lo, hi = c * REAL, min((c + 1) * REAL, N)
        loc = np.arange(hi - lo)
        rho = np.where(loc < HALF - 1, loc + 1, loc + 2)
        xs[rho] = x[lo:hi]
        in_maps.append({
            "xT": np.ascontiguousarray(xs.T),
            "W1": W1, "W2": W2, "b1bc": b1bc, "b2bc": b2bc,
            "woutbc": woutbc, "boutbc": boutbc,
            "deg": deg_tiles[c], "gidx": gidx_all[c], "cidx": cidx_all[c],
            "ident": ident,
        })

    res = bass_utils.run_bass_kernel_spmd(
        nc, in_maps, core_ids=list(range(NCORES)), trace=_trace)

    out = np.zeros(N, np.float32)
    for c in range(NCORES):
        o = res.results[c]["out"]
        arr = o.T.ravel()
        lo, hi = c * REAL, min((c + 1) * REAL, N)
        loc = np.arange(hi - lo)
        rho = np.where(loc < HALF - 1, loc + 1, loc + 2)
        out[lo:hi] = arr[rho]
    if _trace:
        return out, res.exec_time_ns
    return out
